# revision 28
# baseline (speedup 1.0000x reference)
"""Trainium2 Bass kernel for AttentionFlowLayer scores.

S[b,t,j] = C[b,t,:]@wC + Q[b,j,:]@wQ + sum_d C[b,t,d]*wCmQ[d]*Q[b,j,d] + bias

Full shapes: C [64,2048,128] f32, Q [64,512,128] f32 -> S [64,2048,512] f32.
Data-parallel over batch across 8 NeuronCores (8 batches per core).

HBM-bandwidth-bound problem (per-core traffic sets the floor), so all
device I/O is bf16: inputs are cast + pre-transposed to [d, t]/[d, j] on
the host (layout prep only — zero FLOPs), the output S is written bf16
and upcast on the host. Measured end-to-end rel_l2 ~3e-3.

Per core (software-pipelined over its 8 batches):
  - DMA ct[d=128, t=2048] / qt[d=128, j=512] bf16 per batch; both are
    fully contiguous per partition in DRAM (host pre-transposed, t-axis
    permuted so output partitions land on contiguous DRAM rows), so no
    on-device transposes at all.
  - p1 folded into the matmul: r[d,j] = qt[d,j]*wCmQ[d] + wC[d], so
    ct_tile.T @ r = p3 + p1 (each row of r carries +wC[d]).
  - p2+bias replicated across partitions by a const-weight matmul
    (wq outer ones) @ qt, then ACT Identity(+bias) PSUM->SBUF bf16.
  - Tiles processed in PAIRS sharing a 2-bank PSUM tile [128,1024] so
    each epilogue instruction covers 1024 elems (halves fixed overhead).
    Per-pair epilogue class balances DVE/ACT/GPS (k=1 matmul p2-fold
    was tried and is a trap: 1-row matmuls don't register as PE-array
    activity, HAM keeps the PE at 1.2GHz and every MM runs isolated):
      D: DVE tensor_add(PSUM2, p2 bcast) -> bf16   (PSUM f32 = 1x DVE)
      C: ACT Identity PSUM2->bf16, DVE bf16 add    (bf16 SBUF = 2x DVE)
      H: ACT Identity PSUM2->bf16, GPS bf16 add
  - All store DMAs ride the two HWDGE rings (sync=SP, scalar=ACT)
    so GPSIMD's Q7 never does store descriptor generation and has
    capacity for the H-class adds + r-prep.
  - Output staged in [128, GSZ*512] bf16 groups; half-group DMAs (sync
    queue) write GSZ/2 consecutive t-rows per partition contiguously.
"""

import os
import sys

for _p in ("/opt/trn_rl_repo", "/opt/pypackages"):
    if _p not in sys.path and os.path.isdir(_p):
        sys.path.append(_p)

import numpy as np

import concourse.bass as bass
import concourse.mybir as mybir
import concourse.tile as tile
from concourse import bacc
from concourse.bass import ds, ts
from concourse.bass_utils import run_bass_kernel_spmd

F32 = mybir.dt.float32
BF16 = mybir.dt.bfloat16
AF = mybir.ActivationFunctionType
ALU = mybir.AluOpType

N_CORES = 8
B_FULL, T, D = 64, 2048, 128
J = 512
B_LOC = B_FULL // N_CORES  # 8 batches per core
N_TTILE = T // 128  # 16
N_PAIR = N_TTILE // 2  # 8 pairs per batch

# Epilogue pair-class mix per 8 pairs (rest use the C path).
P_D = int(os.environ.get("KERNEL_PD", "3"))  # DVE add from 2-bank PSUM
P_H = int(os.environ.get("KERNEL_PH", "1"))  # ACT copy + GpSimd bf16 add
GSZ = int(os.environ.get("KERNEL_GSZ", "8"))  # tiles per output group
assert N_TTILE % GSZ == 0 and GSZ % 2 == 0


def _pair_classes():
    """Interleave the epilogue classes evenly across the 8 pairs."""
    counts = {"d": P_D, "h": P_H, "c": N_PAIR - P_D - P_H}
    assert counts["c"] >= 0
    classes = []
    rem = dict(counts)
    for i in range(N_PAIR):
        k = max(rem, key=lambda x: rem[x])
        classes.append(k)
        rem[k] -= 1
    return classes


def _build_nc():
    nc = bacc.Bacc("TRN2", target_bir_lowering=False, debug=False,
                   num_devices=N_CORES)
    C_d = nc.dram_tensor("C_t", [B_LOC, D, T], BF16, kind="ExternalInput")
    Q_d = nc.dram_tensor("Q_t", [B_LOC, D, J], BF16, kind="ExternalInput")
    w3_d = nc.dram_tensor("w3_col", [128, 3], F32, kind="ExternalInput")
    wqo_d = nc.dram_tensor("wq_ones", [128, 128], BF16, kind="ExternalInput")
    S_d = nc.dram_tensor("S_s", [B_LOC, T, J], BF16, kind="ExternalOutput")

    classes = _pair_classes()

    import contextlib
    stack = contextlib.ExitStack()
    with tile.TileContext(nc) as tc, stack:
        const_pool = stack.enter_context(tc.tile_pool(name="const", bufs=1))
        ct_pool = stack.enter_context(tc.tile_pool(name="ct", bufs=4))
        qt_pool = stack.enter_context(tc.tile_pool(name="qt", bufs=4))
        qside_pool = stack.enter_context(tc.tile_pool(name="qside", bufs=4))
        p2d_pool = stack.enter_context(tc.tile_pool(name="p2d", bufs=2))
        tmp_pool = stack.enter_context(tc.tile_pool(name="tmp", bufs=3))
        out_pool = stack.enter_context(tc.tile_pool(name="outsb", bufs=3))
        ps_s = stack.enter_context(tc.tile_pool(name="ps_s", bufs=3,
                                                space="PSUM"))
        ps_p2 = stack.enter_context(tc.tile_pool(name="ps_p2", bufs=2,
                                                 space="PSUM"))

        # Constants ride the otherwise-idle gpsimd queue in one [128,3]
        # tensor so batch 0's r-prep isn't gated by the scalar queue's
        # ACT table load + per-DMA descriptor generation.
        w3_sb = const_pool.tile([128, 3], F32, name="w3_sb")
        nc.gpsimd.dma_start(w3_sb[:], w3_d.ap())
        wc_sb = w3_sb[:, 0:1]
        wcmq_sb = w3_sb[:, 1:2]
        bias_sb = w3_sb[:, 2:3]
        wqo_sb = const_pool.tile([128, 128], BF16, name="wqo_sb")
        nc.gpsimd.dma_start(wqo_sb[:], wqo_d.ap())

        C_ap = C_d.ap()
        Q_ap = Q_d.ap()
        S_ap = S_d.ap()

        st = {}  # per-batch live tiles

        def emit_load(b):
            qt = qt_pool.tile([128, J], BF16, name="qt", tag="qt")
            nc.sync.dma_start(qt[:], Q_ap[b])
            ct = ct_pool.tile([128, T], BF16, name="ct", tag="ct")
            if b == 0:
                # split so the first pair's weights land sooner
                nc.sync.dma_start(ct[:, 0:T // 2], C_ap[b][:, 0:T // 2])
                nc.sync.dma_start(ct[:, T // 2:T], C_ap[b][:, T // 2:T])
            else:
                nc.sync.dma_start(ct[:], C_ap[b])
            st[b] = {"ct": ct, "qt": qt}

        def emit_qprep(b):
            s = st[b]
            # r[d,j] = qt*wcmq + wc. DVE for batch 0 (shortens the head
            # while DVE is idle), gpsimd afterwards (keeps DVE free).
            eng = nc.vector if b == 0 else nc.gpsimd
            r = qside_pool.tile([128, J], BF16, name="r", tag="r")
            eng.tensor_scalar(r[:], s["qt"][:], wcmq_sb,
                              wc_sb, ALU.mult, ALU.add)
            s["r"] = r
            # p2 replicated over partitions: (wQ outer ones) @ qt
            p2ps = ps_p2.tile([128, J], F32, name="p2ps", tag="p2ps")
            nc.tensor.matmul(p2ps[:], wqo_sb[:], s["qt"][:],
                             start=True, stop=True)
            p2rep = qside_pool.tile([128, J], BF16, name="p2rep", tag="p2rep")
            nc.scalar.activation(p2rep[:], p2ps[:], AF.Identity,
                                 bias=bias_sb)
            s["p2rep"] = p2rep
            # duplicate p2 side-by-side so pair epilogues use DENSE
            # [128,1024] operands: stride-0 broadcast APs disqualify
            # DVE's packed 2x uop; dense bf16 keeps the fast path.
            p2d = p2d_pool.tile([128, 2 * J], BF16, name="p2d", tag="p2d")
            nc.vector.tensor_copy(p2d[:, 0:J], p2rep[:])
            nc.vector.tensor_copy(p2d[:, J:2 * J], p2rep[:])
            s["p2d"] = p2d

        def emit_pair(b, pi):
            s = st[b]
            cls = classes[pi]
            i0 = 2 * pi
            ps2 = ps_s.tile([128, 2 * J], F32, name="ps2", tag="ps2")
            nc.tensor.matmul(ps2[:, 0:J], s["ct"][:, ts(i0, 128)], s["r"][:],
                             start=True, stop=True)
            nc.tensor.matmul(ps2[:, J:2 * J], s["ct"][:, ts(i0 + 1, 128)],
                             s["r"][:], start=True, stop=True)
            if i0 % GSZ == 0:
                s["outg"] = out_pool.tile([128, GSZ * J], BF16, name="outg",
                                          tag="outg")
            out2 = s["outg"][:, ds((i0 % GSZ) * J, 2 * J)]
            if cls == "d":
                nc.vector.tensor_add(out2, ps2[:], s["p2d"][:])
            else:
                tmp = tmp_pool.tile([128, 2 * J], BF16, name="tmp", tag="tmp")
                nc.scalar.activation(tmp[:], ps2[:], AF.Identity)
                eng = nc.gpsimd if cls == "h" else nc.vector
                eng.tensor_add(out2, tmp[:], s["p2d"][:])
            # store each half-group as soon as its slots are done; both
            # HWDGE rings (sync=SP, scalar=ACT) alternate so one ring's
            # FIFO never paces the stores and GPSIMD does no desc-gen.
            half = GSZ // 2
            i1 = i0 + 1
            if i1 % half == half - 1:
                h = i1 // half
                eng = nc.sync if (b * 4 + h) % 2 == 0 else nc.gpsimd
                eng.dma_start(
                    S_ap[b].rearrange("(p k) j -> p k j", k=16)[
                        :, ds(half * h, half), :],
                    s["outg"][:].rearrange("p (k j) -> p k j", j=J)[
                        :, ds(half * (h % 2), half), :])

        def emit_release(b):
            st.pop(b, None)

        # Software pipeline: loads ride 2 batches ahead (the input stream
        # doubles as filler traffic between compute-paced store bursts),
        # qprep 1 ahead.
        emit_load(0)
        emit_load(1)
        emit_qprep(0)
        for b in range(B_LOC):
            for pi in range(N_PAIR):
                emit_pair(b, pi)
                if pi == 0 and b + 2 < B_LOC:
                    emit_load(b + 2)
                if pi == 1 and b + 1 < B_LOC:
                    emit_qprep(b + 1)
            emit_release(b)

    nc.compile()
    return nc


_NC_CACHE = None


def _get_nc():
    global _NC_CACHE
    if _NC_CACHE is None:
        _NC_CACHE = _build_nc()
    return _NC_CACHE


def _make_in_maps(C, Q, weight_C, weight_Q, weight_CmQ, bias):
    import ml_dtypes
    bf = ml_dtypes.bfloat16
    C = np.asarray(C, dtype=np.float32)
    Q = np.asarray(Q, dtype=np.float32)
    wc = np.asarray(weight_C, dtype=np.float32).reshape(128, 1)
    wq = np.asarray(weight_Q, dtype=np.float32).reshape(128, 1)
    wcmq = np.asarray(weight_CmQ, dtype=np.float32).reshape(128, 1)
    bias_rep = np.full((128, 1), float(np.asarray(bias).reshape(-1)[0]),
                       dtype=np.float32)
    wq_ones = np.ascontiguousarray(np.tile(wq, (1, 128)).astype(bf))
    w3 = np.ascontiguousarray(np.concatenate([wc, wcmq, bias_rep], axis=1))
    # bf16 + [d, t]/[d, j] layout: d on partitions, per-partition rows
    # contiguous in DRAM. The t axis is permuted so that out-tile i's
    # partition p lands on DRAM row t = 16p + (i//GSZ)*GSZ + i%GSZ, making
    # each output group's DMA write GSZ consecutive rows per partition.
    i_idx = np.arange(N_TTILE).repeat(128)
    p_idx = np.tile(np.arange(128), N_TTILE)
    t_perm = 16 * p_idx + (i_idx // GSZ) * GSZ + (i_idx % GSZ)
    C_t = np.ascontiguousarray(C[:, t_perm, :].transpose(0, 2, 1).astype(bf))
    Q_t = np.ascontiguousarray(Q.transpose(0, 2, 1).astype(bf))
    in_maps = []
    for k in range(N_CORES):
        in_maps.append({
            "C_t": np.ascontiguousarray(C_t[k * B_LOC:(k + 1) * B_LOC]),
            "Q_t": np.ascontiguousarray(Q_t[k * B_LOC:(k + 1) * B_LOC]),
            "w3_col": w3,
            "wq_ones": wq_ones,
        })
    return in_maps


def _run(in_maps, **kw):
    nc = _get_nc()
    return run_bass_kernel_spmd(nc, in_maps, core_ids=list(range(N_CORES)), **kw)


def kernel(C, Q, weight_C, weight_Q, weight_CmQ, bias):
    in_maps = _make_in_maps(C, Q, weight_C, weight_Q, weight_CmQ, bias)
    res = _run(in_maps)
    return np.concatenate(
        [r["S_s"].astype(np.float32) for r in res.results], axis=0)


def _install_ntff_hook():
    """Provide antenv.axon_hooks (absent on this image) backed by the
    libaxon_pjrt.so NRT-profile C ABI, so trace=True works under axon."""
    import types
    if "antenv.axon_hooks" in sys.modules:
        return
    try:
        from trn_agent_boot.trn_boot import _ntff_profile_via_ctypes
        hook = _ntff_profile_via_ctypes("/opt/axon/libaxon_pjrt.so")
    except Exception:
        hook = None
    mod = types.ModuleType("antenv.axon_hooks")
    _state = {"hook": hook}
    mod.set_axon_ntff_profile_hook = lambda h: _state.__setitem__("hook", h)
    mod.get_axon_ntff_profile_hook = lambda: _state["hook"]
    sys.modules["antenv.axon_hooks"] = mod


def kernel_traced(C, Q, weight_C, weight_Q, weight_CmQ, bias, **kw):
    """Like kernel() but with NTFF tracing; returns (out, BassKernelResults)."""
    _install_ntff_hook()
    in_maps = _make_in_maps(C, Q, weight_C, weight_Q, weight_CmQ, bias)
    res = _run(in_maps, trace=True, **kw)
    out = np.concatenate(
        [r["S_s"].astype(np.float32) for r in res.results], axis=0)
    return out, res



# revision 32
# speedup vs baseline: 1.1727x; 1.1727x over previous
"""Trainium2 Bass kernel for AttentionFlowLayer scores.

S[b,t,j] = C[b,t,:]@wC + Q[b,j,:]@wQ + sum_d C[b,t,d]*wCmQ[d]*Q[b,j,d] + bias

Full shapes: C [64,2048,128] f32, Q [64,512,128] f32 -> S [64,2048,512] f32.
Data-parallel over batch across 8 NeuronCores (8 batches per core).

HBM-bandwidth-bound problem (per-core traffic sets the floor), so all
device I/O is bf16: inputs are cast + pre-transposed to [d, t]/[d, j] on
the host (layout prep only — zero FLOPs), the output S is written bf16
and upcast on the host. Measured end-to-end rel_l2 ~3e-3.

Per core (software-pipelined over its 8 batches):
  - DMA ct[d=128, t=2048] / qt[d=128, j=512] bf16 per batch; both are
    fully contiguous per partition in DRAM (host pre-transposed, t-axis
    permuted so output partitions land on contiguous DRAM rows), so no
    on-device transposes at all.
  - p1 folded into the matmul: r[d,j] = qt[d,j]*wCmQ[d] + wC[d], so
    ct_tile.T @ r = p3 + p1 (each row of r carries +wC[d]).
  - p2+bias replicated across partitions by a const-weight matmul
    (wq outer ones) @ qt, then ACT Identity(+bias) PSUM->SBUF bf16.
  - Tiles processed in PAIRS sharing a 2-bank PSUM tile [128,1024] so
    each epilogue instruction covers 1024 elems (halves fixed overhead).
    Per-pair epilogue class balances DVE/ACT/GPS (k=1 matmul p2-fold
    was tried and is a trap: 1-row matmuls don't register as PE-array
    activity, HAM keeps the PE at 1.2GHz and every MM runs isolated):
      D: DVE tensor_add(PSUM2, p2 bcast) -> bf16   (PSUM f32 = 1x DVE)
      C: ACT Identity PSUM2->bf16, DVE bf16 add    (bf16 SBUF = 2x DVE)
      H: ACT Identity PSUM2->bf16, GPS bf16 add
  - All store DMAs ride the two HWDGE rings (sync=SP, scalar=ACT)
    so GPSIMD's Q7 never does store descriptor generation and has
    capacity for the H-class adds + r-prep.
  - Output staged in [128, GSZ*512] bf16 groups; half-group DMAs (sync
    queue) write GSZ/2 consecutive t-rows per partition contiguously.
"""

import os
import sys

for _p in ("/opt/trn_rl_repo", "/opt/pypackages"):
    if _p not in sys.path and os.path.isdir(_p):
        sys.path.append(_p)

import numpy as np

import concourse.bass as bass
import concourse.mybir as mybir
import concourse.tile as tile
from concourse import bacc
from concourse.bass import ds, ts
from concourse.bass_utils import run_bass_kernel_spmd

F32 = mybir.dt.float32
BF16 = mybir.dt.bfloat16
AF = mybir.ActivationFunctionType
ALU = mybir.AluOpType

N_CORES = 8
B_FULL, T, D = 64, 2048, 128
J = 512
B_LOC = B_FULL // N_CORES  # 8 batches per core
N_TTILE = T // 128  # 16
N_PAIR = N_TTILE // 2  # 8 pairs per batch

# Epilogue pair-class mix per 8 pairs (rest use the C path).
P_D = int(os.environ.get("KERNEL_PD", "3"))  # DVE add from 2-bank PSUM
P_H = int(os.environ.get("KERNEL_PH", "1"))  # ACT copy + GpSimd bf16 add
GSZ = int(os.environ.get("KERNEL_GSZ", "8"))  # tiles per output group
assert N_TTILE % GSZ == 0 and GSZ % 2 == 0


def _pair_classes():
    """Interleave the epilogue classes evenly across the 8 pairs."""
    counts = {"d": P_D, "h": P_H, "c": N_PAIR - P_D - P_H}
    assert counts["c"] >= 0
    classes = []
    rem = dict(counts)
    for i in range(N_PAIR):
        k = max(rem, key=lambda x: rem[x])
        classes.append(k)
        rem[k] -= 1
    return classes


def _build_nc():
    nc = bacc.Bacc("TRN2", target_bir_lowering=False, debug=False,
                   num_devices=N_CORES)
    C_d = nc.dram_tensor("C_t", [B_LOC, D, T], BF16, kind="ExternalInput")
    Q_d = nc.dram_tensor("Q_t", [B_LOC, D, J], BF16, kind="ExternalInput")
    w3_d = nc.dram_tensor("w3_col", [128, 3], F32, kind="ExternalInput")
    wqo_d = nc.dram_tensor("wq_ones", [128, 128], BF16, kind="ExternalInput")
    S_d = nc.dram_tensor("S_s", [B_LOC, T, J], BF16, kind="ExternalOutput")

    classes = _pair_classes()

    import contextlib
    stack = contextlib.ExitStack()
    with tile.TileContext(nc) as tc, stack:
        const_pool = stack.enter_context(tc.tile_pool(name="const", bufs=1))
        ct_pool = stack.enter_context(tc.tile_pool(name="ct", bufs=4))
        qt_pool = stack.enter_context(tc.tile_pool(name="qt", bufs=4))
        qside_pool = stack.enter_context(tc.tile_pool(name="qside", bufs=4))
        tmp_pool = stack.enter_context(tc.tile_pool(name="tmp", bufs=3))
        out_pool = stack.enter_context(tc.tile_pool(name="outsb", bufs=3))
        ps_s = stack.enter_context(tc.tile_pool(name="ps_s", bufs=3,
                                                space="PSUM"))
        ps_p2 = stack.enter_context(tc.tile_pool(name="ps_p2", bufs=2,
                                                 space="PSUM"))

        # Constants ride the otherwise-idle gpsimd queue in one [128,3]
        # tensor so batch 0's r-prep isn't gated by the scalar queue's
        # ACT table load + per-DMA descriptor generation.
        w3_sb = const_pool.tile([128, 3], F32, name="w3_sb")
        nc.gpsimd.dma_start(w3_sb[:], w3_d.ap())
        wc_sb = w3_sb[:, 0:1]
        wcmq_sb = w3_sb[:, 1:2]
        bias_sb = w3_sb[:, 2:3]
        wqo_sb = const_pool.tile([128, 128], BF16, name="wqo_sb")
        nc.gpsimd.dma_start(wqo_sb[:], wqo_d.ap())

        C_ap = C_d.ap()
        Q_ap = Q_d.ap()
        S_ap = S_d.ap()

        st = {}  # per-batch live tiles

        def emit_load(b):
            qt = qt_pool.tile([128, J], BF16, name="qt", tag="qt")
            nc.sync.dma_start(qt[:], Q_ap[b])
            ct = ct_pool.tile([128, T], BF16, name="ct", tag="ct")
            if b == 0:
                # split so the first pair's weights land sooner
                nc.sync.dma_start(ct[:, 0:T // 2], C_ap[b][:, 0:T // 2])
                nc.sync.dma_start(ct[:, T // 2:T], C_ap[b][:, T // 2:T])
            else:
                nc.sync.dma_start(ct[:], C_ap[b])
            st[b] = {"ct": ct, "qt": qt}

        def emit_qprep(b):
            s = st[b]
            # r[d,j] = qt*wcmq + wc. DVE for batch 0 (shortens the head
            # while DVE is idle), gpsimd afterwards (keeps DVE free).
            eng = nc.vector if b == 0 else nc.gpsimd
            r = qside_pool.tile([128, J], BF16, name="r", tag="r")
            eng.tensor_scalar(r[:], s["qt"][:], wcmq_sb,
                              wc_sb, ALU.mult, ALU.add)
            s["r"] = r
            # p2 replicated over partitions: (wQ outer ones) @ qt
            p2ps = ps_p2.tile([128, J], F32, name="p2ps", tag="p2ps")
            nc.tensor.matmul(p2ps[:], wqo_sb[:], s["qt"][:],
                             start=True, stop=True)
            p2rep = qside_pool.tile([128, J], BF16, name="p2rep", tag="p2rep")
            nc.scalar.activation(p2rep[:], p2ps[:], AF.Identity,
                                 bias=bias_sb)
            s["p2rep"] = p2rep

        def _as3d(ap):
            return ap.rearrange("p (k j) -> p k j", j=J)

        def emit_pair(b, pi):
            s = st[b]
            cls = classes[pi]
            i0 = 2 * pi
            ps2 = ps_s.tile([128, 2 * J], F32, name="ps2", tag="ps2")
            nc.tensor.matmul(ps2[:, 0:J], s["ct"][:, ts(i0, 128)], s["r"][:],
                             start=True, stop=True)
            nc.tensor.matmul(ps2[:, J:2 * J], s["ct"][:, ts(i0 + 1, 128)],
                             s["r"][:], start=True, stop=True)
            if i0 % GSZ == 0:
                s["outg"] = out_pool.tile([128, GSZ * J], BF16, name="outg",
                                          tag="outg")
            out2 = s["outg"][:, ds((i0 % GSZ) * J, 2 * J)]
            p2b = s["p2rep"][:].unsqueeze(1).broadcast_to([128, 2, J])
            if cls == "d":
                nc.vector.tensor_add(_as3d(out2), _as3d(ps2[:]), p2b)
            else:
                tmp = tmp_pool.tile([128, 2 * J], BF16, name="tmp", tag="tmp")
                nc.scalar.activation(tmp[:], ps2[:], AF.Identity)
                eng = nc.gpsimd if cls == "h" else nc.vector
                eng.tensor_add(_as3d(out2), _as3d(tmp[:]), p2b)
            # store each half-group as soon as its slots are done; both
            # HWDGE rings (sync=SP, scalar=ACT) alternate so one ring's
            # FIFO never paces the stores and GPSIMD does no desc-gen.
            half = GSZ // 2
            i1 = i0 + 1
            if i1 % half == half - 1:
                h = i1 // half
                eng = nc.sync if (b * 4 + h) % 2 == 0 else nc.gpsimd
                eng.dma_start(
                    S_ap[b].rearrange("(p k) j -> p k j", k=16)[
                        :, ds(half * h, half), :],
                    s["outg"][:].rearrange("p (k j) -> p k j", j=J)[
                        :, ds(half * (h % 2), half), :])

        def emit_release(b):
            st.pop(b, None)

        # Software pipeline: loads ride 2 batches ahead (the input stream
        # doubles as filler traffic between compute-paced store bursts),
        # qprep 1 ahead.
        emit_load(0)
        emit_load(1)
        emit_qprep(0)
        for b in range(B_LOC):
            for pi in range(N_PAIR):
                emit_pair(b, pi)
                if pi == 0 and b + 2 < B_LOC:
                    emit_load(b + 2)
                if pi == 1 and b + 1 < B_LOC:
                    emit_qprep(b + 1)
            emit_release(b)

    nc.compile()
    return nc


_NC_CACHE = None


def _get_nc():
    global _NC_CACHE
    if _NC_CACHE is None:
        _NC_CACHE = _build_nc()
    return _NC_CACHE


def _make_in_maps(C, Q, weight_C, weight_Q, weight_CmQ, bias):
    import ml_dtypes
    bf = ml_dtypes.bfloat16
    C = np.asarray(C, dtype=np.float32)
    Q = np.asarray(Q, dtype=np.float32)
    wc = np.asarray(weight_C, dtype=np.float32).reshape(128, 1)
    wq = np.asarray(weight_Q, dtype=np.float32).reshape(128, 1)
    wcmq = np.asarray(weight_CmQ, dtype=np.float32).reshape(128, 1)
    bias_rep = np.full((128, 1), float(np.asarray(bias).reshape(-1)[0]),
                       dtype=np.float32)
    wq_ones = np.ascontiguousarray(np.tile(wq, (1, 128)).astype(bf))
    w3 = np.ascontiguousarray(np.concatenate([wc, wcmq, bias_rep], axis=1))
    # bf16 + [d, t]/[d, j] layout: d on partitions, per-partition rows
    # contiguous in DRAM. The t axis is permuted so that out-tile i's
    # partition p lands on DRAM row t = 16p + (i//GSZ)*GSZ + i%GSZ, making
    # each output group's DMA write GSZ consecutive rows per partition.
    i_idx = np.arange(N_TTILE).repeat(128)
    p_idx = np.tile(np.arange(128), N_TTILE)
    t_perm = 16 * p_idx + (i_idx // GSZ) * GSZ + (i_idx % GSZ)
    C_t = np.ascontiguousarray(C[:, t_perm, :].transpose(0, 2, 1).astype(bf))
    Q_t = np.ascontiguousarray(Q.transpose(0, 2, 1).astype(bf))
    in_maps = []
    for k in range(N_CORES):
        in_maps.append({
            "C_t": np.ascontiguousarray(C_t[k * B_LOC:(k + 1) * B_LOC]),
            "Q_t": np.ascontiguousarray(Q_t[k * B_LOC:(k + 1) * B_LOC]),
            "w3_col": w3,
            "wq_ones": wq_ones,
        })
    return in_maps


def _run(in_maps, **kw):
    nc = _get_nc()
    return run_bass_kernel_spmd(nc, in_maps, core_ids=list(range(N_CORES)), **kw)


def kernel(C, Q, weight_C, weight_Q, weight_CmQ, bias):
    in_maps = _make_in_maps(C, Q, weight_C, weight_Q, weight_CmQ, bias)
    res = _run(in_maps)
    return np.concatenate(
        [r["S_s"].astype(np.float32) for r in res.results], axis=0)


def _install_ntff_hook():
    """Provide antenv.axon_hooks (absent on this image) backed by the
    libaxon_pjrt.so NRT-profile C ABI, so trace=True works under axon."""
    import types
    if "antenv.axon_hooks" in sys.modules:
        return
    try:
        from trn_agent_boot.trn_boot import _ntff_profile_via_ctypes
        hook = _ntff_profile_via_ctypes("/opt/axon/libaxon_pjrt.so")
    except Exception:
        hook = None
    mod = types.ModuleType("antenv.axon_hooks")
    _state = {"hook": hook}
    mod.set_axon_ntff_profile_hook = lambda h: _state.__setitem__("hook", h)
    mod.get_axon_ntff_profile_hook = lambda: _state["hook"]
    sys.modules["antenv.axon_hooks"] = mod


def kernel_traced(C, Q, weight_C, weight_Q, weight_CmQ, bias, **kw):
    """Like kernel() but with NTFF tracing; returns (out, BassKernelResults)."""
    _install_ntff_hook()
    in_maps = _make_in_maps(C, Q, weight_C, weight_Q, weight_CmQ, bias)
    res = _run(in_maps, trace=True, **kw)
    out = np.concatenate(
        [r["S_s"].astype(np.float32) for r in res.results], axis=0)
    return out, res

